# revision 13
# baseline (speedup 1.0000x reference)
"""CrossAttn + TISA bias kernel for TRN2, 8-core SPMD.

Sharding: core = (batch b = core//2, query half = core%2).
Each core computes the full kv projection for its batch (duplicated within
the pair) and its 512 query rows end-to-end. No collectives.

v2 vs v1:
  - softmax denominator fused into the attn matmul: stationary [v_h | 1]
    (M=65) so each wT tile streams through the PE once, not twice
  - per-head reciprocal broadcast via a K=1 ones matmul (bf16)
  - odd heads' normalized attn moved to partitions 64..127 with a small
    SBUF->SBUF DMA (engines cannot shift partitions)
  - weight DMAs reordered/column-chunked so the first q-proj matmul starts
    after ~0.4 MB instead of 9 MB
  - gate phase emits 512-col output halves, sigmoid path first, so the
    tail after the last matmul is short

Inputs arrive host-transposed: xqt/xkvt are [d_in, tokens].
  qT:   [d_out(part), i]   (scaled by 1/sqrt(Dh))
  kT:   [d_out(part), j]
  vaug: [j(part), jc, h, 65] = v columns 0..63, ones column 64
  S^T:  [j(part), i] = kT_h.T @ qT_h          (K=64)
  wT = exp(S^T) * srow[:, C:C+512]            (shifted exp-bias table slice)
  ps_h = [vaug_h]^T @ wT accumulated over jc  -> rows 0..63 attn, row 64 sums
  rb = ones1^T @ (1/sums)                     (K=1 PE broadcast)
  attn = ps_h * rb                            (per-head normalize)
  gate: attn.T @ Wg -> [i(part), 2048]; out = (a+bga)*sigmoid(b+bgb)
"""

import numpy as np
import ml_dtypes

import concourse.bacc as bacc
import concourse.mybir as mybir
import concourse.tile as tile
from concourse.bass import ts

L = 1024
D = 1024
H = 16
DH = 64
LQ = 512          # q rows per core
NIC = LQ // 128   # 4 i-chunks
NJC = L // 128    # 8 j-chunks
NKC = D // 128    # 8 d_model chunks
SROW_W = 1408
NUM_KERNELS = 21

F32 = mybir.dt.float32
BF16 = mybir.dt.bfloat16
EXP = mybir.ActivationFunctionType.Exp
SIG = mybir.ActivationFunctionType.Sigmoid
CPY = mybir.ActivationFunctionType.Copy
MUL = mybir.AluOpType.mult
ADD = mybir.AluOpType.add

_NP = {"f32": np.float32, "bf16": ml_dtypes.bfloat16}


def ds2(hh):
    return slice(hh * 64, hh * 64 + 64)


def build_nc(cfg="bf16"):
    mdt = BF16

    nc = bacc.Bacc("TRN2", target_bir_lowering=False, debug=False, num_devices=8)

    xqt_d = nc.dram_tensor("xqt", [D, LQ], mdt, kind="ExternalInput").ap()
    xkvt_d = nc.dram_tensor("xkvt", [D, L], mdt, kind="ExternalInput").ap()
    wqc_d = nc.dram_tensor("wqc", [NKC, D, 128], mdt, kind="ExternalInput").ap()
    wmk_d = nc.dram_tensor("wmk", [NKC, D, 128], mdt, kind="ExternalInput").ap()
    wmv_d = nc.dram_tensor("wmv", [D, D], mdt, kind="ExternalInput").ap()
    wg_d = nc.dram_tensor("wg", [D, 2 * D], mdt, kind="ExternalInput").ap()
    srow_d = nc.dram_tensor("srow", [H, 128, SROW_W], mdt, kind="ExternalInput").ap()
    bg_d = nc.dram_tensor("bgrep", [128, 2 * D], F32, kind="ExternalInput").ap()
    out_d = nc.dram_tensor("out", [LQ, D], F32, kind="ExternalOutput").ap()

    with tile.TileContext(nc) as tc:
        with (
            tc.tile_pool(name="const", bufs=1) as constp,
            tc.tile_pool(name="persist", bufs=1) as pers,
            tc.tile_pool(name="psum", bufs=1, space="PSUM") as psum,
        ):
            onesc = constp.tile([128, 64], mdt)
            nc.gpsimd.memset(onesc, 1.0)

            # PE warmup: keep the array busy during the initial input DMA so
            # the first real matmuls run at full clock (pstate ramp).
            dummy = constp.tile([128, 512], mdt)
            nc.gpsimd.memset(dummy, 0.0)
            for _ in range(10):
                ps_w = psum.tile([128, 512], F32, tag="t1", bufs=2)
                nc.tensor.matmul(ps_w, dummy[:, 0:128], dummy, start=True, stop=True)

            qT = pers.tile([128, NKC, LQ], mdt)        # [d_out, mc, i]
            kT = pers.tile([128, NKC, L], mdt)         # [d_out, mc, j]
            vaug = pers.tile([128, NJC, H, 65], mdt)   # [j, jc, h, v|1]
            attn = pers.tile([128, NKC, LQ], mdt)      # [d_model, chunk, i]

            # =========== phase B: projections ==========
            with tc.tile_pool(name="phB", bufs=1) as phb:
                wqc = phb.tile([128, NKC, NKC, 128], mdt)   # [k, mc, kc, col]
                xqT = phb.tile([128, NKC, LQ], mdt)         # [d_in, kc, i]
                xkvT = phb.tile([128, NKC, L], mdt)         # [d_in, kc, j]
                wmk = phb.tile([128, NKC, NKC, 128], mdt)
                wmv = phb.tile([128, NKC, D], mdt)          # [k, kc, v-cols]

                # Weights stream on the Pool SWDGE queue; activations on the
                # SP HWDGE queue. The two queues issue in parallel so the
                # first q-proj matmul starts after ~0.4 MB, not 9 MB.
                for mc in range(NKC):
                    nc.gpsimd.dma_start(
                        out=wqc[:, mc],
                        in_=wqc_d[mc].rearrange("(kc p) c -> p kc c", p=128))
                for mc in range(NKC):
                    nc.gpsimd.dma_start(
                        out=wmk[:, mc],
                        in_=wmk_d[mc].rearrange("(kc p) c -> p kc c", p=128))
                for kc in range(NKC):
                    nc.gpsimd.dma_start(out=wmv[:, kc, :], in_=wmv_d[ts(kc, 128), :])
                for kc in range(NKC):
                    nc.sync.dma_start(out=xqT[:, kc, :], in_=xqt_d[ts(kc, 128), :])
                for kc in range(NKC):
                    # Act HWDGE queue: keeps SP free for xqT (needed first)
                    nc.scalar.dma_start(out=xkvT[:, kc, :], in_=xkvt_d[ts(kc, 128), :])

                for jc in range(NJC):
                    nc.gpsimd.memset(vaug[:, jc, :, 64], 1.0)

                # q projection
                for mc in range(NKC):
                    ps = psum.tile([128, LQ], F32, tag="t1", bufs=2)
                    for kc in range(NKC):
                        nc.tensor.matmul(
                            ps, wqc[:, mc, kc, :], xqT[:, kc, :],
                            start=(kc == 0), stop=(kc == NKC - 1))
                    nc.scalar.activation(qT[:, mc, :], ps, CPY, scale=0.125)

                # k projection
                for mc in range(NKC):
                    for nh in range(2):
                        ps = psum.tile([128, 512], F32, tag="t1", bufs=2)
                        for kc in range(NKC):
                            nc.tensor.matmul(
                                ps, wmk[:, mc, kc, :], xkvT[:, kc, ts(nh, 512)],
                                start=(kc == 0), stop=(kc == NKC - 1))
                        if nh == 0:
                            nc.vector.tensor_copy(kT[:, mc, ts(nh, 512)], ps)
                        else:
                            nc.scalar.activation(kT[:, mc, ts(nh, 512)], ps, CPY)

                # v projection -> strided into vaug (cols 0..63 per head)
                for jc in range(NJC):
                    for nh in range(2):
                        ps = psum.tile([128, 512], F32, tag="t1", bufs=2)
                        for kc in range(NKC):
                            nc.tensor.matmul(
                                ps, xkvT[:, kc, ts(jc, 128)], wmv[:, kc, ts(nh, 512)],
                                start=(kc == 0), stop=(kc == NKC - 1))
                        dst = vaug[:, jc, nh * 8:(nh + 1) * 8, 0:64]
                        if nh == 0:
                            nc.vector.tensor_copy(dst, ps)
                        else:
                            nc.scalar.activation(dst, ps, CPY)

            # ================= phase C: attention =================
            with tc.tile_pool(name="phC", bufs=1) as phc:
                # gate-phase weights load during C (Pool SWDGE queue)
                wg_r = phc.tile([128, NKC, 2 * D], mdt)
                bg_sb = phc.tile([128, 2 * D], F32)
                for kc in range(NKC):
                    nc.gpsimd.dma_start(out=wg_r[:, kc, :], in_=wg_d[ts(kc, 128), :])
                nc.gpsimd.dma_start(out=bg_sb, in_=bg_d)

                # srow prefetch, 4 heads deep (SP queue drains early)
                srow_tiles = {}
                for h in range(4):
                    srow_tiles[h] = phc.tile([128, SROW_W], mdt, tag="srow", bufs=4, name=f"srow{h}")
                    nc.sync.dma_start(out=srow_tiles[h], in_=srow_d[h, :, :])

                for c in range(NKC):
                    for hh in range(2):
                        h = 2 * c + hh
                        srow_sb = srow_tiles.pop(h)
                        if h + 4 < H:
                            srow_tiles[h + 4] = phc.tile(
                                [128, SROW_W], mdt, tag="srow", bufs=4,
                                name=f"srow{h + 4}")
                            nc.sync.dma_start(
                                out=srow_tiles[h + 4], in_=srow_d[h + 4, :, :])
                        ps_h = psum.tile([65, LQ], F32, tag="psh", bufs=2)
                        for jp in range(NJC // 2):
                            # scores for a jc pair share a 2-bank psum so one
                            # EXP covers 1024 cols (halves Act fixed cost)
                            ps_s = psum.tile([128, 2 * LQ], F32, tag="ss", bufs=2)
                            for t in range(2):
                                jc = 2 * jp + t
                                nc.tensor.matmul(
                                    ps_s[:, ts(t, LQ)],
                                    kT[ds2(hh), c, ts(jc, 128)], qT[ds2(hh), c, :],
                                    start=True, stop=True)
                            wexp = phc.tile([128, 2 * LQ], mdt, tag="wexp", bufs=3)
                            nc.scalar.activation(wexp, ps_s, EXP)
                            for t in range(2):
                                jc = 2 * jp + t
                                wT = phc.tile([128, LQ], mdt, tag="wt", bufs=6)
                                C0 = 896 - jc * 128
                                # all-SBUF bf16 op: legal on Pool; offload 2
                                # of 8 per head to keep DVE under the Act EXP
                                eng = nc.gpsimd if jc in (3, 7) else nc.vector
                                eng.tensor_tensor(
                                    wT, wexp[:, ts(t, LQ)],
                                    srow_sb[:, C0:C0 + LQ], MUL)
                                nc.tensor.matmul(
                                    ps_h, vaug[:, jc, h, :], wT,
                                    start=(jc == 0), stop=(jc == NJC - 1))
                        rsb = phc.tile([128, LQ], mdt, tag="rsb", bufs=2)
                        with nc.allow_low_precision(reason="softmax recip bf16"):
                            nc.vector.reciprocal(rsb[64:65, :], ps_h[64:65, :])
                        rb_ps = psum.tile([64, LQ], F32, tag="t1", bufs=2)
                        nc.tensor.matmul(
                            rb_ps, onesc[64:65, :], rsb[64:65, :],
                            start=True, stop=True)
                        rb_sb = phc.tile([64, LQ], F32, tag="rbs", bufs=2)
                        nc.vector.tensor_copy(rb_sb, rb_ps)
                        if hh == 0:
                            nc.vector.tensor_tensor(
                                attn[0:64, c, :], ps_h[0:64, :], rb_sb, MUL)
                        else:
                            todd = phc.tile([64, LQ], mdt, tag="todd", bufs=2)
                            nc.vector.tensor_tensor(todd, ps_h[0:64, :], rb_sb, MUL)
                            nc.sync.dma_start(out=attn[64:128, c, :], in_=todd)

                # ================= phase D: gate =================
                for ic in range(NIC):
                    for qa in range(2):
                        ps_b = psum.tile([128, 512], F32, tag="t1", bufs=2)
                        for kc in range(NKC):
                            nc.tensor.matmul(
                                ps_b, attn[:, kc, ts(ic, 128)],
                                wg_r[:, kc, slice(D + qa * 512, D + qa * 512 + 512)],
                                start=(kc == 0), stop=(kc == NKC - 1))
                        tb = phc.tile([128, 512], F32, tag="tb", bufs=2)
                        nc.vector.tensor_tensor(
                            tb, ps_b, bg_sb[:, D + qa * 512:D + qa * 512 + 512], ADD)
                        tsg = phc.tile([128, 512], F32, tag="tsg", bufs=2)
                        nc.scalar.activation(tsg, tb, SIG)

                        ps_a = psum.tile([128, 512], F32, tag="t1", bufs=2)
                        for kc in range(NKC):
                            nc.tensor.matmul(
                                ps_a, attn[:, kc, ts(ic, 128)],
                                wg_r[:, kc, ts(qa, 512)],
                                start=(kc == 0), stop=(kc == NKC - 1))
                        last = (ic == NIC - 1) and (qa == 1)
                        if not last:
                            ta = phc.tile([128, 512], F32, tag="ta", bufs=2)
                            nc.vector.tensor_tensor(
                                ta, ps_a, bg_sb[:, ts(qa, 512)], ADD)
                            outh = phc.tile([128, 512], F32, tag="outt", bufs=3)
                            nc.vector.tensor_tensor(outh, ta, tsg, MUL)
                            nc.sync.dma_start(
                                out=out_d[ts(ic, 128), ts(qa, 512)], in_=outh)
                        else:
                            # split the final chunk so the post-matmul tail
                            # (adds, mul, DMA) pipelines in 128-col pieces
                            for qt in range(4):
                                sl = slice(qt * 128, qt * 128 + 128)
                                ta = phc.tile([128, 128], F32, tag="ta2", bufs=2)
                                nc.vector.tensor_tensor(
                                    ta, ps_a[:, sl],
                                    bg_sb[:, qa * 512 + qt * 128:
                                          qa * 512 + qt * 128 + 128], ADD)
                                outh = phc.tile([128, 128], F32, tag="outt2", bufs=2)
                                nc.vector.tensor_tensor(
                                    outh, ta, tsg[:, sl], MUL)
                                nc.sync.dma_start(
                                    out=out_d[ts(ic, 128),
                                              qa * 512 + qt * 128:
                                              qa * 512 + qt * 128 + 128],
                                    in_=outh)

    nc.compile()
    return nc


# ======================= host side =======================

def _tisa_ebias(amp, off, sharp):
    d = np.arange(-(L - 1), L, dtype=np.float32)
    s = np.sum(
        amp[:, :, None].astype(np.float32)
        * np.exp(-np.abs(sharp)[:, :, None].astype(np.float32)
                 * (d[None, None, :] - off[:, :, None].astype(np.float32)) ** 2),
        axis=1, dtype=np.float32).astype(np.float32)
    return np.exp(s).astype(np.float32)


def make_host_inputs(inputs, cfg="bf16"):
    npdt = _NP["bf16"]
    x_q = np.asarray(inputs["x_q"])
    x_kv = np.asarray(inputs["x_kv"])
    wq = np.asarray(inputs["Wq"]).astype(npdt)
    wm = np.asarray(inputs["Wm"]).astype(npdt)
    wg = np.asarray(inputs["Wg"]).astype(npdt)
    bg = np.asarray(inputs["bg"]).astype(np.float32)

    ebias = _tisa_ebias(np.asarray(inputs["tisa_amp"]),
                        np.asarray(inputs["tisa_off"]),
                        np.asarray(inputs["tisa_sharp"]))

    p_i = np.arange(128)[:, None]
    m_i = np.arange(SROW_W)[None, :]
    srows = []
    for i_off in (0, 512):
        idx = p_i - m_i + (1919 - i_off)
        srows.append(np.ascontiguousarray(ebias[:, idx]).astype(npdt))

    # column-chunked (mc-major) layouts for early compute start
    wqc = np.ascontiguousarray(
        wq.reshape(D, NKC, 128).transpose(1, 0, 2))          # [mc, k, col]
    wmk = np.ascontiguousarray(
        wm[:, :D].reshape(D, NKC, 128).transpose(1, 0, 2))   # [mc, k, col]
    wmv = np.ascontiguousarray(wm[:, D:])                    # [k, v-col]

    bgrep = np.ascontiguousarray(np.broadcast_to(bg, (128, 2 * D))).astype(np.float32)

    in_maps = []
    for core in range(8):
        b, half = core // 2, core % 2
        in_maps.append({
            "xqt": np.ascontiguousarray(
                x_q[b, half * LQ:(half + 1) * LQ].T).astype(npdt),
            "xkvt": np.ascontiguousarray(x_kv[b].T).astype(npdt),
            "wqc": wqc, "wmk": wmk, "wmv": wmv, "wg": wg,
            "srow": srows[half],
            "bgrep": bgrep,
        })
    return in_maps


def assemble_output(results):
    out = np.empty((4, L, D), dtype=np.float32)
    for core in range(8):
        b, half = core // 2, core % 2
        out[b, half * LQ:(half + 1) * LQ] = results[core]["out"]
    return out


# ======================= public entry point =======================

_NC_CACHE = {}


def _get_nc(cfg):
    if cfg not in _NC_CACHE:
        _NC_CACHE[cfg] = build_nc(cfg)
    return _NC_CACHE[cfg]


def kernel(**inputs):
    """Full (unsharded) inputs -> full (4, 1024, 1024) float32 output.

    Shards over 8 NeuronCores: core = (batch, query-half). Host precomputes
    the TISA exp-bias lookup table and pre-transposes activations; all dense
    compute (projections, attention, gate) runs on-device in bf16 matmuls
    with fp32 accumulation.
    """
    from concourse.bass_utils import run_bass_kernel_spmd

    cfg = "bf16"
    nc = _get_nc(cfg)
    in_maps = make_host_inputs(inputs, cfg)
    res = run_bass_kernel_spmd(nc, in_maps, core_ids=list(range(8)))
    return assemble_output(res.results)
